# revision 8
# baseline (speedup 1.0000x reference)
"""Trainium2 Bass kernel for nn_CrossAttention (B=8, T=1024, TE=256, C=1024, CE=768, H=16).

Sharding: data-parallel over batch — 8 NeuronCores, one batch element each.
No collectives needed; weights are replicated to every core.

Per-core dataflow (one batch element; matmul operand tiles use float32r —
fp32 storage processed at full PE rate for moving dim >= 256):

  host:  xT = x[b].T, encT = enc[b].T, bp2 = bv @ Wp + bp   (bias folding:
         softmax rows sum to 1, so  (A @ (V + 1 bv^T)) Wp + bp = A V Wp + (bv Wp + bp))

  QT [c,t]   = Wq_chunk.T @ xT      (lhsT = Wq natural chunks)   + bq (per-partition)
  KT [c,te]  = Wk_chunk.T @ encT                                 + bk (per-partition)
  V  [te,c]  = encT_chunk.T @ Wv    (natural layout, no bias — folded into bp2)
  S2_h [t,te]  = QT_h.T @ KT_h  (per head, K=64)   -> E2 = exp(S2/8), accum R = rowsum
  A_h = E2 * (1/R) / 16  accumulated over heads -> att_mean  [t,te]
  ST_h [te,t]  = KT_h.T @ QT_h                    -> ET = exp(ST/8)   [te,t]
  U_h [t,hd]   = ET_h.T @ V_h   (K=te, 2 chunks)  -> Y[:,t,h*64:] = U * (1/R)
  YT = PE-transpose(Y)  per 128x128 block
  y [t,c]   = YT_chunk.T @ Wp + bp2
"""

import numpy as np
from contextlib import ExitStack

B, T, TE = 8, 1024, 256
C, CE, H = 1024, 768, 16
HD = C // H  # 64
P = 128
NT = T // P    # 8 t-tiles
NC = C // P    # 8 c-tiles
KC = C // P    # 8 contraction chunks over C
KE = CE // P   # 6 contraction chunks over CE
NTE = TE // P  # 2 te-tiles

_CACHE = {}


def _build_bass(mm_dt_name="float32r"):
    import concourse.tile as tile
    from concourse import mybir, bacc
    from concourse.masks import make_identity

    f32 = mybir.dt.float32
    fr = getattr(mybir.dt, mm_dt_name)
    ab = mybir.dt.bfloat16   # attention-core dtype (scores + PV)

    nc = bacc.Bacc("TRN2", target_bir_lowering=False, debug=False, num_devices=B)

    # Tensors feeding fp32r matmuls are declared float32r end-to-end so every
    # producer (DMA / ACT / DVE) writes properly rounded values — the BIR
    # verifier rejects fp32-written data consumed by an FP32r matmul.
    xT = nc.dram_tensor("xT", [C, T], fr, kind="ExternalInput").ap()
    encT = nc.dram_tensor("encT", [CE, TE], fr, kind="ExternalInput").ap()
    Wq = nc.dram_tensor("Wq", [C, C], fr, kind="ExternalInput").ap()
    Wk = nc.dram_tensor("Wk", [CE, C], fr, kind="ExternalInput").ap()
    Wv = nc.dram_tensor("Wv", [CE, C], fr, kind="ExternalInput").ap()
    Wp = nc.dram_tensor("Wp", [C, C], fr, kind="ExternalInput").ap()
    bq = nc.dram_tensor("bq", [C], f32, kind="ExternalInput").ap()
    bk = nc.dram_tensor("bk", [C], f32, kind="ExternalInput").ap()
    bp2 = nc.dram_tensor("bp2", [C], f32, kind="ExternalInput").ap()
    y = nc.dram_tensor("y", [T, C], f32, kind="ExternalOutput").ap()
    att = nc.dram_tensor("att", [T, TE], f32, kind="ExternalOutput").ap()

    Exp = mybir.ActivationFunctionType.Exp
    ADD = mybir.AluOpType.add
    MUL = mybir.AluOpType.mult

    with tile.TileContext(nc) as tc, ExitStack() as ctx:
        persist = ctx.enter_context(tc.tile_pool(name="persist", bufs=1))
        bigw = ctx.enter_context(tc.tile_pool(name="bigw", bufs=1))
        wpool = ctx.enter_context(tc.tile_pool(name="wpool", bufs=2))
        etpool = ctx.enter_context(tc.tile_pool(name="etpool", bufs=2))
        e2pool = ctx.enter_context(tc.tile_pool(name="e2pool", bufs=10))
        ypool = ctx.enter_context(tc.tile_pool(name="ypool", bufs=3))
        yttp = ctx.enter_context(tc.tile_pool(name="yttp", bufs=2))
        psA = ctx.enter_context(tc.tile_pool(name="psA", bufs=3, space="PSUM"))
        psB = ctx.enter_context(tc.tile_pool(name="psB", bufs=3, space="PSUM"))
        psC = ctx.enter_context(tc.tile_pool(name="psC", bufs=2, space="PSUM"))

        # ---- persistent tiles + loads ----
        xT_r = xT.rearrange("(k p) t -> p k t", p=P)
        xT_sb = bigw.tile([P, KC, T], fr, tag="bigw")
        for k in range(KC):
            nc.sync.dma_start(out=xT_sb[:, k, :], in_=xT_r[:, k, :])
        encT_r = encT.rearrange("(k p) e -> p k e", p=P)
        encT_sb = persist.tile([P, KE, TE], fr)
        for k in range(KE):
            nc.sync.dma_start(out=encT_sb[:, k, :], in_=encT_r[:, k, :])
        bq_sb = persist.tile([P, NC], f32)
        nc.sync.dma_start(out=bq_sb, in_=bq.rearrange("(j p) -> p j", p=P))
        bk_sb = persist.tile([P, NC], f32)
        nc.sync.dma_start(out=bk_sb, in_=bk.rearrange("(j p) -> p j", p=P))
        bp2_bc = persist.tile([P, C], f32)
        nc.sync.dma_start(out=bp2_bc, in_=bp2.partition_broadcast(P))

        ident = persist.tile([P, P], f32)
        make_identity(nc, ident)

        QT_sb = persist.tile([P, NC, T], ab)
        KT_sb = persist.tile([P, NC, TE], ab)
        V_sb = persist.tile([P, NTE, C], ab)
        Y_sb = persist.tile([P, NT, C], f32)
        att_acc = persist.tile([P, NT, TE], f32)
        R_sb = persist.tile([P, NT * H], f32)
        r_sb = persist.tile([P, NT * H], f32)
        r16_sb = persist.tile([P, NT * H], f32)

        Wq_r = Wq.rearrange("(k p) c -> p k c", p=P)
        Wk_r = Wk.rearrange("(k p) c -> p k c", p=P)
        Wv_r = Wv.rearrange("(k p) c -> p k c", p=P)
        Wp_r = Wp.rearrange("(k p) c -> p k c", p=P)

        # ---- Q projection: QT[c, t] ----
        for j in range(NC):
            wq_t = wpool.tile([P, KC, P], fr, tag="w")
            nc.sync.dma_start(out=wq_t, in_=Wq_r[:, :, j * P:(j + 1) * P])
            for nh in range(2):
                ps = psA.tile([P, 512], f32, tag="mm512")
                for k in range(KC):
                    nc.tensor.matmul(
                        ps, lhsT=wq_t[:, k, :],
                        rhs=xT_sb[:, k, nh * 512:(nh + 1) * 512],
                        start=(k == 0), stop=(k == KC - 1),
                    )
                nc.vector.tensor_scalar_add(
                    out=QT_sb[:, j, nh * 512:(nh + 1) * 512], in0=ps,
                    scalar1=bq_sb[:, j:j + 1],
                )

        # ---- K projection: KT[c, te] ----
        for j in range(NC):
            wk_t = wpool.tile([P, KE, P], fr, tag="w")
            nc.sync.dma_start(out=wk_t, in_=Wk_r[:, :, j * P:(j + 1) * P])
            ps = psB.tile([P, TE], f32, tag="ps256")
            for k in range(KE):
                nc.tensor.matmul(
                    ps, lhsT=wk_t[:, k, :], rhs=encT_sb[:, k, :],
                    start=(k == 0), stop=(k == KE - 1),
                )
            nc.vector.tensor_scalar_add(
                out=KT_sb[:, j, :], in0=ps, scalar1=bk_sb[:, j:j + 1],
            )

        # ---- V projection: V[te, c] (no bias; folded into bp2) ----
        for q4 in range(4):
            wv_t = wpool.tile([P, KE, 256], fr, tag="w")
            nc.sync.dma_start(out=wv_t, in_=Wv_r[:, :, q4 * 256:(q4 + 1) * 256])
            for tt2 in range(NTE):
                ps = psB.tile([P, 256], f32, tag="ps256")
                for k in range(KE):
                    nc.tensor.matmul(
                        ps, lhsT=encT_sb[:, k, tt2 * P:(tt2 + 1) * P],
                        rhs=wv_t[:, k, :],
                        start=(k == 0), stop=(k == KE - 1),
                    )
                nc.any.tensor_copy(
                    out=V_sb[:, tt2, q4 * 256:(q4 + 1) * 256], in_=ps)

        # ---- per head: S2+softmax stats+att | ST -> ET -> U -> Y ----
        Rv = R_sb.rearrange("p (t h2) -> p h2 t", h2=H)
        rv = r_sb.rearrange("p (t h2) -> p h2 t", h2=H)
        r16v = r16_sb.rearrange("p (t h2) -> p h2 t", h2=H)
        for h in range(H):
            pb, j = (h % 2) * HD, h // 2
            et = etpool.tile([P, NTE, T], ab, tag="et")
            for tt2 in range(NTE):
                for nh in range(2):
                    ps = psA.tile([P, 512], f32, tag="mm512")
                    nc.tensor.matmul(
                        ps, lhsT=KT_sb[pb:pb + HD, j, tt2 * P:(tt2 + 1) * P],
                        rhs=QT_sb[pb:pb + HD, j, nh * 512:(nh + 1) * 512],
                        start=True, stop=True,
                    )
                    nc.scalar.activation(
                        out=et[:, tt2, nh * 512:(nh + 1) * 512], in_=ps,
                        func=Exp, scale=0.125,
                    )
            # S2 scores [t, te] for this head over all t-tiles
            e2_tiles = []
            for tt in range(NT):
                ps = psB.tile([P, TE], f32, tag="ps256")
                nc.tensor.matmul(
                    ps, lhsT=QT_sb[pb:pb + HD, j, tt * P:(tt + 1) * P],
                    rhs=KT_sb[pb:pb + HD, j, :],
                    start=True, stop=True,
                )
                e2 = e2pool.tile([P, TE], ab, tag="e2")
                nc.scalar.activation(
                    out=e2, in_=ps, func=Exp, scale=0.125,
                    accum_out=R_sb[:, tt * H + h:tt * H + h + 1],
                )
                e2_tiles.append(e2)
            nc.vector.reciprocal(out=rv[:, h, :], in_=Rv[:, h, :])
            nc.vector.tensor_scalar_mul(
                out=r16v[:, h, :], in0=rv[:, h, :], scalar1=1.0 / H)
            for tt in range(NT):
                if h == 0:
                    nc.vector.tensor_scalar_mul(
                        out=att_acc[:, tt, :], in0=e2_tiles[tt],
                        scalar1=r16_sb[:, tt * H + h:tt * H + h + 1],
                    )
                else:
                    a_tmp = e2pool.tile([P, TE], ab, tag="a")
                    nc.vector.tensor_scalar_mul(
                        out=a_tmp, in0=e2_tiles[tt],
                        scalar1=r16_sb[:, tt * H + h:tt * H + h + 1],
                    )
                    nc.gpsimd.tensor_tensor(
                        out=att_acc[:, tt, :], in0=att_acc[:, tt, :],
                        in1=a_tmp, op=ADD,
                    )
            for tt in range(NT):
                ps = psC.tile([P, HD], f32, tag="small")
                for tt2 in range(NTE):
                    nc.tensor.matmul(
                        ps, lhsT=et[:, tt2, tt * P:(tt + 1) * P],
                        rhs=V_sb[:, tt2, h * HD:(h + 1) * HD],
                        start=(tt2 == 0), stop=(tt2 == NTE - 1),
                    )
                nc.vector.tensor_scalar_mul(
                    out=Y_sb[:, tt, h * HD:(h + 1) * HD], in0=ps,
                    scalar1=r_sb[:, tt * H + h:tt * H + h + 1],
                )

        # ---- load Wp (slot freed by xT) ----
        Wp_sb = bigw.tile([P, KC, C], fr, tag="bigw")
        for k in range(KC):
            nc.sync.dma_start(out=Wp_sb[:, k, :], in_=Wp_r[:, k, :])

        # ---- transpose Y per t-tile, then project ----
        for tt in range(NT):
            ytt = yttp.tile([P, NC, P], fr, tag="ytt")
            for jj in range(NC):
                ps = psC.tile([P, P], f32, tag="small")
                nc.tensor.transpose(
                    out=ps, in_=Y_sb[:, tt, jj * P:(jj + 1) * P], identity=ident)
                nc.vector.tensor_copy(out=ytt[:, jj, :], in_=ps)
            for nh in range(2):
                ps = psA.tile([P, 512], f32, tag="mm512")
                for k in range(KC):
                    nc.tensor.matmul(
                        ps, lhsT=ytt[:, k, :],
                        rhs=Wp_sb[:, k, nh * 512:(nh + 1) * 512],
                        start=(k == 0), stop=(k == KC - 1),
                    )
                yout = ypool.tile([P, 512], f32, tag="yout")
                nc.vector.tensor_tensor(
                    out=yout, in0=ps, in1=bp2_bc[:, nh * 512:(nh + 1) * 512],
                    op=ADD,
                )
                nc.sync.dma_start(
                    out=y[tt * P:(tt + 1) * P, nh * 512:(nh + 1) * 512], in_=yout)
            nc.sync.dma_start(out=att[tt * P:(tt + 1) * P, :], in_=att_acc[:, tt, :])

    nc.compile()
    return nc


def _get_nc(mm_dt_name="float32r"):
    key = ("nc", mm_dt_name)
    if key not in _CACHE:
        _CACHE[key] = _build_bass(mm_dt_name)
    return _CACHE[key]


def kernel(x, encoder_output, Wq, bq, Wk, bk, Wv, bv, Wp, bp, _trace=False):
    from concourse.bass_utils import run_bass_kernel_spmd

    x = np.asarray(x, dtype=np.float32)
    encoder_output = np.asarray(encoder_output, dtype=np.float32)
    Wq = np.ascontiguousarray(np.asarray(Wq, dtype=np.float32))
    Wk = np.ascontiguousarray(np.asarray(Wk, dtype=np.float32))
    Wv = np.ascontiguousarray(np.asarray(Wv, dtype=np.float32))
    Wp = np.ascontiguousarray(np.asarray(Wp, dtype=np.float32))
    bq = np.ascontiguousarray(np.asarray(bq, dtype=np.float32))
    bk = np.ascontiguousarray(np.asarray(bk, dtype=np.float32))
    bp2 = (np.asarray(bv, dtype=np.float64) @ np.asarray(Wp, dtype=np.float64)
           + np.asarray(bp, dtype=np.float64)).astype(np.float32)

    nc = _get_nc()
    in_maps = []
    for b in range(B):
        in_maps.append({
            "xT": np.ascontiguousarray(x[b].T),
            "encT": np.ascontiguousarray(encoder_output[b].T),
            "Wq": Wq, "Wk": Wk, "Wv": Wv, "Wp": Wp,
            "bq": bq, "bk": bk, "bp2": bp2,
        })
    res = run_bass_kernel_spmd(nc, in_maps, list(range(B)), trace=_trace)
    y = np.stack([res.results[b]["y"] for b in range(B)])
    att = np.stack([res.results[b]["att"] for b in range(B)])
    if _trace:
        return (y, att), res
    return y, att


# revision 9
# speedup vs baseline: 1.1038x; 1.1038x over previous
"""Trainium2 Bass kernel for nn_CrossAttention (B=8, T=1024, TE=256, C=1024, CE=768, H=16).

Sharding: data-parallel over batch — 8 NeuronCores, one batch element each.
No collectives needed; weights are replicated to every core.

Per-core dataflow (one batch element; matmul operand tiles use float32r —
fp32 storage processed at full PE rate for moving dim >= 256):

  host:  xT = x[b].T, encT = enc[b].T, bp2 = bv @ Wp + bp   (bias folding:
         softmax rows sum to 1, so  (A @ (V + 1 bv^T)) Wp + bp = A V Wp + (bv Wp + bp))

  QT [c,t]   = Wq_chunk.T @ xT      (lhsT = Wq natural chunks)   + bq (per-partition)
  KT [c,te]  = Wk_chunk.T @ encT                                 + bk (per-partition)
  V  [te,c]  = encT_chunk.T @ Wv    (natural layout, no bias — folded into bp2)
  S2_h [t,te]  = QT_h.T @ KT_h  (per head, K=64)   -> E2 = exp(S2/8), accum R = rowsum
  A_h = E2 * (1/R) / 16  accumulated over heads -> att_mean  [t,te]
  ST_h [te,t]  = KT_h.T @ QT_h                    -> ET = exp(ST/8)   [te,t]
  U_h [t,hd]   = ET_h.T @ V_h   (K=te, 2 chunks)  -> Y[:,t,h*64:] = U * (1/R)
  YT = PE-transpose(Y)  per 128x128 block
  y [t,c]   = YT_chunk.T @ Wp + bp2
"""

import numpy as np
from contextlib import ExitStack

B, T, TE = 8, 1024, 256
C, CE, H = 1024, 768, 16
HD = C // H  # 64
P = 128
NT = T // P    # 8 t-tiles
NC = C // P    # 8 c-tiles
KC = C // P    # 8 contraction chunks over C
KE = CE // P   # 6 contraction chunks over CE
NTE = TE // P  # 2 te-tiles

_CACHE = {}


def _build_bass(mm_dt_name="float32r"):
    import concourse.tile as tile
    from concourse import mybir, bacc
    from concourse.masks import make_identity

    f32 = mybir.dt.float32
    fr = getattr(mybir.dt, mm_dt_name)
    ab = mybir.dt.bfloat16   # attention-core dtype (scores + PV)

    nc = bacc.Bacc("TRN2", target_bir_lowering=False, debug=False, num_devices=B)

    # Tensors feeding fp32r matmuls are declared float32r end-to-end so every
    # producer (DMA / ACT / DVE) writes properly rounded values — the BIR
    # verifier rejects fp32-written data consumed by an FP32r matmul.
    xT = nc.dram_tensor("xT", [C, T], fr, kind="ExternalInput").ap()
    encT = nc.dram_tensor("encT", [CE, TE], fr, kind="ExternalInput").ap()
    Wq = nc.dram_tensor("Wq", [C, C], fr, kind="ExternalInput").ap()
    Wk = nc.dram_tensor("Wk", [CE, C], fr, kind="ExternalInput").ap()
    Wv = nc.dram_tensor("Wv", [CE, C], fr, kind="ExternalInput").ap()
    Wp = nc.dram_tensor("Wp", [C, C], fr, kind="ExternalInput").ap()
    bq = nc.dram_tensor("bq", [C], f32, kind="ExternalInput").ap()
    bk = nc.dram_tensor("bk", [C], f32, kind="ExternalInput").ap()
    bp2 = nc.dram_tensor("bp2", [C], f32, kind="ExternalInput").ap()
    y = nc.dram_tensor("y", [T, C], f32, kind="ExternalOutput").ap()
    att = nc.dram_tensor("att", [T, TE], f32, kind="ExternalOutput").ap()

    Exp = mybir.ActivationFunctionType.Exp
    ADD = mybir.AluOpType.add
    MUL = mybir.AluOpType.mult

    with tile.TileContext(nc) as tc, ExitStack() as ctx:
        persist = ctx.enter_context(tc.tile_pool(name="persist", bufs=1))
        bigw = ctx.enter_context(tc.tile_pool(name="bigw", bufs=1))
        wpool = ctx.enter_context(tc.tile_pool(name="wpool", bufs=2))
        etpool = ctx.enter_context(tc.tile_pool(name="etpool", bufs=2))
        e2pool = ctx.enter_context(tc.tile_pool(name="e2pool", bufs=10))
        ypool = ctx.enter_context(tc.tile_pool(name="ypool", bufs=3))
        yttp = ctx.enter_context(tc.tile_pool(name="yttp", bufs=2))
        psA = ctx.enter_context(tc.tile_pool(name="psA", bufs=3, space="PSUM"))
        psB = ctx.enter_context(tc.tile_pool(name="psB", bufs=3, space="PSUM"))
        psC = ctx.enter_context(tc.tile_pool(name="psC", bufs=2, space="PSUM"))

        # ---- persistent tiles + loads ----
        xT_r = xT.rearrange("(k p) t -> p k t", p=P)
        xT_sb = bigw.tile([P, KC, T], fr, tag="bigw")
        for k in range(KC):
            nc.sync.dma_start(out=xT_sb[:, k, :], in_=xT_r[:, k, :])
        encT_r = encT.rearrange("(k p) e -> p k e", p=P)
        encT_sb = persist.tile([P, KE, TE], fr)
        for k in range(KE):
            nc.sync.dma_start(out=encT_sb[:, k, :], in_=encT_r[:, k, :])
        bq_sb = persist.tile([P, NC], f32)
        nc.sync.dma_start(out=bq_sb, in_=bq.rearrange("(j p) -> p j", p=P))
        bk_sb = persist.tile([P, NC], f32)
        nc.sync.dma_start(out=bk_sb, in_=bk.rearrange("(j p) -> p j", p=P))
        bp2_bc = persist.tile([P, C], f32)
        nc.sync.dma_start(out=bp2_bc, in_=bp2.partition_broadcast(P))

        ident = persist.tile([P, P], f32)
        make_identity(nc, ident)

        QT_sb = persist.tile([P, NC, T], ab)
        KT_sb = persist.tile([P, NC, TE], ab)
        V_sb = persist.tile([P, NTE, C], ab)
        Y_sb = persist.tile([P, NT, C], f32)
        att_acc = persist.tile([P, NT, TE], f32)
        R_sb = persist.tile([P, NT * H], f32)
        r_sb = persist.tile([P, NT * H], f32)
        r16_sb = persist.tile([P, NT * H], f32)

        Wq_r = Wq.rearrange("(k p) c -> p k c", p=P)
        Wk_r = Wk.rearrange("(k p) c -> p k c", p=P)
        Wv_r = Wv.rearrange("(k p) c -> p k c", p=P)
        Wp_r = Wp.rearrange("(k p) c -> p k c", p=P)

        # ---- Q projection: QT[c, t] ----
        for j in range(NC):
            wq_t = wpool.tile([P, KC, P], fr, tag="w")
            nc.sync.dma_start(out=wq_t, in_=Wq_r[:, :, j * P:(j + 1) * P])
            for nh in range(2):
                ps = psA.tile([P, 512], f32, tag="mm512")
                for k in range(KC):
                    nc.tensor.matmul(
                        ps, lhsT=wq_t[:, k, :],
                        rhs=xT_sb[:, k, nh * 512:(nh + 1) * 512],
                        start=(k == 0), stop=(k == KC - 1),
                    )
                nc.vector.tensor_scalar_add(
                    out=QT_sb[:, j, nh * 512:(nh + 1) * 512], in0=ps,
                    scalar1=bq_sb[:, j:j + 1],
                )

        # ---- K projection: KT[c, te] ----
        for j in range(NC):
            wk_t = wpool.tile([P, KE, P], fr, tag="w")
            nc.sync.dma_start(out=wk_t, in_=Wk_r[:, :, j * P:(j + 1) * P])
            ps = psB.tile([P, TE], f32, tag="ps256")
            for k in range(KE):
                nc.tensor.matmul(
                    ps, lhsT=wk_t[:, k, :], rhs=encT_sb[:, k, :],
                    start=(k == 0), stop=(k == KE - 1),
                )
            nc.vector.tensor_scalar_add(
                out=KT_sb[:, j, :], in0=ps, scalar1=bk_sb[:, j:j + 1],
            )

        # ---- V projection: V[te, c] (no bias; folded into bp2) ----
        for q4 in range(4):
            wv_t = wpool.tile([P, KE, 256], fr, tag="w")
            nc.sync.dma_start(out=wv_t, in_=Wv_r[:, :, q4 * 256:(q4 + 1) * 256])
            for tt2 in range(NTE):
                ps = psB.tile([P, 256], f32, tag="ps256")
                for k in range(KE):
                    nc.tensor.matmul(
                        ps, lhsT=encT_sb[:, k, tt2 * P:(tt2 + 1) * P],
                        rhs=wv_t[:, k, :],
                        start=(k == 0), stop=(k == KE - 1),
                    )
                nc.any.tensor_copy(
                    out=V_sb[:, tt2, q4 * 256:(q4 + 1) * 256], in_=ps)

        # ---- S2 path: per (tt, h) scores [t, te], softmax stats + att_mean ----
        for tt in range(NT):
            e2_tiles = {}
            for h in range(H):
                pb, j = (h % 2) * HD, h // 2
                ps = psB.tile([P, TE], f32, tag="ps256")
                nc.tensor.matmul(
                    ps, lhsT=QT_sb[pb:pb + HD, j, tt * P:(tt + 1) * P],
                    rhs=KT_sb[pb:pb + HD, j, :],
                    start=True, stop=True,
                )
                e2 = e2pool.tile([P, TE], ab, tag="e2")
                nc.scalar.activation(
                    out=e2, in_=ps, func=Exp, scale=0.125,
                    accum_out=R_sb[:, tt * H + h:tt * H + h + 1],
                )
                e2_tiles[h] = e2
                if h % 4 == 3:
                    lo = tt * H + h - 3
                    nc.vector.reciprocal(
                        out=r_sb[:, lo:lo + 4], in_=R_sb[:, lo:lo + 4])
                    nc.vector.tensor_scalar_mul(
                        out=r16_sb[:, lo:lo + 4], in0=r_sb[:, lo:lo + 4],
                        scalar1=1.0 / H)
                    for hh in range(h - 3, h + 1):
                        if hh == 0:
                            nc.vector.tensor_scalar_mul(
                                out=att_acc[:, tt, :], in0=e2_tiles[hh],
                                scalar1=r16_sb[:, tt * H + hh:tt * H + hh + 1],
                            )
                        else:
                            a_tmp = e2pool.tile([P, TE], ab, tag="a")
                            nc.vector.tensor_scalar_mul(
                                out=a_tmp, in0=e2_tiles[hh],
                                scalar1=r16_sb[:, tt * H + hh:tt * H + hh + 1],
                            )
                            nc.gpsimd.tensor_tensor(
                                out=att_acc[:, tt, :], in0=att_acc[:, tt, :],
                                in1=a_tmp, op=ADD,
                            )
                        del e2_tiles[hh]

        # ---- attention per head: ST -> ET -> U -> Y ----
        for h in range(H):
            pb, j = (h % 2) * HD, h // 2
            et = etpool.tile([P, NTE, T], ab, tag="et")
            for tt2 in range(NTE):
                for nh in range(2):
                    ps = psA.tile([P, 512], f32, tag="mm512")
                    nc.tensor.matmul(
                        ps, lhsT=KT_sb[pb:pb + HD, j, tt2 * P:(tt2 + 1) * P],
                        rhs=QT_sb[pb:pb + HD, j, nh * 512:(nh + 1) * 512],
                        start=True, stop=True,
                    )
                    nc.scalar.activation(
                        out=et[:, tt2, nh * 512:(nh + 1) * 512], in_=ps,
                        func=Exp, scale=0.125,
                    )
            for tt in range(NT):
                ps = psC.tile([P, HD], f32, tag="small")
                for tt2 in range(NTE):
                    nc.tensor.matmul(
                        ps, lhsT=et[:, tt2, tt * P:(tt + 1) * P],
                        rhs=V_sb[:, tt2, h * HD:(h + 1) * HD],
                        start=(tt2 == 0), stop=(tt2 == NTE - 1),
                    )
                nc.vector.tensor_scalar_mul(
                    out=Y_sb[:, tt, h * HD:(h + 1) * HD], in0=ps,
                    scalar1=r_sb[:, tt * H + h:tt * H + h + 1],
                )

        # ---- load Wp (slot freed by xT) ----
        Wp_sb = bigw.tile([P, KC, C], fr, tag="bigw")
        for k in range(KC):
            nc.sync.dma_start(out=Wp_sb[:, k, :], in_=Wp_r[:, k, :])

        # ---- transpose Y per t-tile, then project ----
        for tt in range(NT):
            ytt = yttp.tile([P, NC, P], fr, tag="ytt")
            for jj in range(NC):
                ps = psC.tile([P, P], f32, tag="small")
                nc.tensor.transpose(
                    out=ps, in_=Y_sb[:, tt, jj * P:(jj + 1) * P], identity=ident)
                nc.vector.tensor_copy(out=ytt[:, jj, :], in_=ps)
            for nh in range(2):
                ps = psA.tile([P, 512], f32, tag="mm512")
                for k in range(KC):
                    nc.tensor.matmul(
                        ps, lhsT=ytt[:, k, :],
                        rhs=Wp_sb[:, k, nh * 512:(nh + 1) * 512],
                        start=(k == 0), stop=(k == KC - 1),
                    )
                yout = ypool.tile([P, 512], f32, tag="yout")
                nc.vector.tensor_tensor(
                    out=yout, in0=ps, in1=bp2_bc[:, nh * 512:(nh + 1) * 512],
                    op=ADD,
                )
                nc.sync.dma_start(
                    out=y[tt * P:(tt + 1) * P, nh * 512:(nh + 1) * 512], in_=yout)
            nc.sync.dma_start(out=att[tt * P:(tt + 1) * P, :], in_=att_acc[:, tt, :])

    nc.compile()
    return nc


def _get_nc(mm_dt_name="float32r"):
    key = ("nc", mm_dt_name)
    if key not in _CACHE:
        _CACHE[key] = _build_bass(mm_dt_name)
    return _CACHE[key]


def kernel(x, encoder_output, Wq, bq, Wk, bk, Wv, bv, Wp, bp, _trace=False):
    from concourse.bass_utils import run_bass_kernel_spmd

    x = np.asarray(x, dtype=np.float32)
    encoder_output = np.asarray(encoder_output, dtype=np.float32)
    Wq = np.ascontiguousarray(np.asarray(Wq, dtype=np.float32))
    Wk = np.ascontiguousarray(np.asarray(Wk, dtype=np.float32))
    Wv = np.ascontiguousarray(np.asarray(Wv, dtype=np.float32))
    Wp = np.ascontiguousarray(np.asarray(Wp, dtype=np.float32))
    bq = np.ascontiguousarray(np.asarray(bq, dtype=np.float32))
    bk = np.ascontiguousarray(np.asarray(bk, dtype=np.float32))
    bp2 = (np.asarray(bv, dtype=np.float64) @ np.asarray(Wp, dtype=np.float64)
           + np.asarray(bp, dtype=np.float64)).astype(np.float32)

    nc = _get_nc()
    in_maps = []
    for b in range(B):
        in_maps.append({
            "xT": np.ascontiguousarray(x[b].T),
            "encT": np.ascontiguousarray(encoder_output[b].T),
            "Wq": Wq, "Wk": Wk, "Wv": Wv, "Wp": Wp,
            "bq": bq, "bk": bk, "bp2": bp2,
        })
    res = run_bass_kernel_spmd(nc, in_maps, list(range(B)), trace=_trace)
    y = np.stack([res.results[b]["y"] for b in range(B)])
    att = np.stack([res.results[b]["att"] for b in range(B)])
    if _trace:
        return (y, att), res
    return y, att
